# revision 25
# baseline (speedup 1.0000x reference)
"""Multi-head attention Trainium2 Bass kernel (v3, fp16/f32r).

Problem: x:(4,512,1024), Wq/Wk/Wv/Wo:(512,512), H=8 heads, d=64.
  q = Wq@x ; k = Wk@x ; v = Wv@x  (1x1 conv == channel matmul)
  per head: S[i,j] = q[:,i].k[:,j] ; attn = softmax_j(S) ; y = attn @ v
  out = Wo @ y

Sharding: 8 cores = (batch b, head-half g).  Core (b,g) handles batch b,
local heads g*4..g*4+3 and computes the partial output
out_p = Wo[:, g*256:(g+1)*256] @ y_g which the host sums pairwise.

Design notes (v3):
- Inputs/projections in fp16 (host-cast): half the DMA of f32, 1
  row/cycle on the PE, and an 11-bit mantissa so score differences stay
  accurate (bf16 cost 1.3e-2 rel err; fp16 ~1e-3).  exp output e and v
  stay f32r (range to e^52 and ACT writes f32 faster than bf16).
- Scores are computed transposed (S^T = k^T q) per head pair with the
  two head lanes ROW-TILED onto PE row groups 0-63 / 64-127 (auto
  tile_position) so both K=64 matmuls run concurrently.
- The ACT-engine exp stream (32 x [128,1024] ~ 36us) is the floor; the
  emission order keeps it dense: block A (pair0/i0) interleaves the
  just-in-time V projection and the pair0 nn=1 QK groups, block B
  (pair0/i1) the pair1 QK projection, block D (pair1/i1) the i0 output
  projection.  Each block's softmax-denominator/normalize chain is
  deferred into the next block so it never stalls the PE FIFO.
- PV keeps the ones-column trick (M=65): row 64 of the PV accumulator
  is the denominator.  It is broadcast across partitions by two K=1
  col-tiled matmuls (rhs = the denominator rows in SBUF), reciprocal on
  DVE, normalize on GPSIMD into a pair-stacked y layout [128, n]
  (head 2g on partitions 0-63, head 2g+1 on 64-127, via a partition-
  shifting PSUM->SBUF copy).
- With y pair-stacked, the output projection is a plain K=128 matmul
  per (m, g): the contraction sums the head pair directly.
"""

import numpy as np

import concourse.bass as bass
import concourse.tile as tile
from concourse import bacc
from concourse import mybir
from concourse.bass_utils import run_bass_kernel_spmd

F32 = mybir.dt.float32
F32R = mybir.dt.float32r
FP16 = mybir.dt.float16

P = 128
C = 512          # channels
NSEQ = 1024      # sequence length
D = 64           # head dim
HL = 4           # local heads per core
KC = C // P      # 4 contraction tiles over channels
J = NSEQ // P    # 8 key tiles

_NC_CACHE = {}


def build_nc():
    nc = bacc.Bacc("TRN2")

    x = nc.dram_tensor("x", [C, NSEQ], FP16, kind="ExternalInput")
    wqkv = nc.dram_tensor("wqkv_t", [C, 3, 2 * P], FP16, kind="ExternalInput")
    # paired Wo: [d + 64*(h%2), h//2, c_out]
    wo = nc.dram_tensor("wo_p", [P, 2, C], FP16, kind="ExternalInput")
    out = nc.dram_tensor("out_p", [C, NSEQ], FP16, kind="ExternalOutput")

    with tile.TileContext(nc) as tc:
        with (
            tc.tile_pool(name="consts", bufs=1) as consts,
            tc.tile_pool(name="epool", bufs=5) as epool,
            tc.tile_pool(name="ypool", bufs=3) as ypool,
            tc.tile_pool(name="rpool", bufs=2) as rpool,
            tc.tile_pool(name="opool", bufs=2) as opool,
            tc.tile_pool(name="pp", bufs=2, space="PSUM") as pp,
        ):
            # ---- persistent tiles
            warm = consts.tile([P, 512], FP16)
            # selector for the denominator broadcast: a K=33 matmul with
            # lhsT rows 0/32 routes lane0's denom to partitions 0-63 and
            # lane1's to 64-127 in one (0,0)-positioned matmul.
            sel = consts.tile([33, P], F32R)
            dd = consts.tile([33, 512], F32R)
            w_sb = [consts.tile([P, 3, 2 * P], FP16, name=f"w{kc}")
                    for kc in range(KC)]
            x_sb = [[consts.tile([P, 512], FP16, name=f"x{kc}_{sh}")
                     for sh in range(2)] for kc in range(KC)]
            wot_sb = consts.tile([P, 2, C], FP16)
            q_sb = [[consts.tile([P, 512], FP16, name=f"q{m}_{sh}")
                     for sh in range(2)] for m in range(2)]
            k_sb = [[consts.tile([P, 512], FP16, name=f"k{m}_{sh}")
                     for sh in range(2)] for m in range(2)]
            vt_sb = [consts.tile([P, HL, D + 1], F32R, name=f"vt{j}")
                     for j in range(J)]
            # pair-stacked y: yt[g][64*lane + d, n] = y_{head 2g+lane}[d, n]
            yt = [consts.tile([P, NSEQ], FP16, name=f"yt{g}")
                  for g in range(2)]

            # ---- init constants (gpsimd; keeps DVE/ACT free)
            nc.gpsimd.memset(warm, 0)
            # memset can't target f32r; write the f32 bit pattern of 1.0
            # through a uint32 view instead.
            ONE_F32 = 0x3F800000
            nc.gpsimd.memset(sel.bitcast(mybir.dt.uint32), 0)
            nc.gpsimd.memset(sel[0:1, 0:D].bitcast(mybir.dt.uint32), ONE_F32)
            nc.gpsimd.memset(sel[32:33, D:2 * D].bitcast(mybir.dt.uint32),
                             ONE_F32)
            # rows 1-31 of dd are never written; zero them once so the
            # 0-weight selector rows can't hit NaNs in the K=33 matmul.
            nc.gpsimd.memset(dd.bitcast(mybir.dt.uint32), 0)
            for j in range(J):
                nc.gpsimd.memset(
                    vt_sb[j][:, :, D:D + 1].bitcast(mybir.dt.uint32), ONE_F32)

            def po_tile(name="po", tag="po"):
                return pp.tile([P, 512], F32, tag=tag, name=name, bufs=2)

            # ---- warm-up matmuls release the HAM clock gate during load
            for wi in range(12):
                nc.tensor.matmul(po_tile("pw"), lhsT=warm[:, 0:P],
                                 rhs=warm, start=True, stop=True)

            # ---- input DMAs, alternating dispatch engines (SP/ACT) so
            # descriptor writes don't serialize; order = first-needed.
            w_t = wqkv.rearrange("(kc p) w m -> p kc w m", p=P)
            x_t = x.rearrange("(kc p) (sh n) -> p kc sh n", p=P, n=512)
            loads = []
            for kc in range(KC):
                loads.append((w_sb[kc], w_t[:, kc]))
            for kc in range(KC):
                loads.append((x_sb[kc][0], x_t[:, kc, 0]))

            def emit_loads(lds):
                for i, (dst, src) in enumerate(lds):
                    eng = nc.sync if i % 2 == 0 else nc.scalar
                    eng.dma_start(dst, src)

            emit_loads(loads)
            loads2 = [(x_sb[kc][1], x_t[:, kc, 1]) for kc in range(KC)]
            loads2.append((wot_sb, wo[:, :, :]))

            # ---- QK projections: group = (w_idx, m, nn), 4 matmuls
            def proj_group(w_idx, m, nn, split=None):
                """split=None: all 4 kc matmuls + copy. split=0: first 2.
                split=1: last 2 + copy."""
                key = (w_idx, m, nn)
                if split in (None, 0):
                    _proj_ps[key] = po_tile()
                ps = _proj_ps[key]
                kcs = range(KC) if split is None else (
                    (0, 1) if split == 0 else (2, 3))
                for kc in kcs:
                    nc.tensor.matmul(
                        ps,
                        lhsT=w_sb[kc][:, w_idx, m * P:(m + 1) * P],
                        rhs=x_sb[kc][nn],
                        start=(kc == 0),
                        stop=(kc == KC - 1),
                    )
                if split in (None, 1):
                    dst = (q_sb if w_idx == 0 else k_sb)[m][nn]
                    nc.vector.tensor_copy(out=dst, in_=ps)

            _proj_ps = {}
            proj_group(0, 0, 0)     # q pair0, seq-half 0
            proj_group(1, 0, 0)     # k pair0, seq-half 0
            emit_loads(loads2)      # seq-half 1 + wo after the first projs

            def vproj(j):
                psv = po_tile()
                for kc in range(KC):
                    nc.tensor.matmul(
                        psv[:, 0:2 * P],
                        lhsT=x_sb[kc][j // 4][:, (j % 4) * P:(j % 4 + 1) * P],
                        rhs=w_sb[kc][:, 2, :],
                        start=(kc == 0),
                        stop=(kc == KC - 1),
                    )
                nc.vector.tensor_copy(
                    out=vt_sb[j][:, :, 0:D],
                    in_=psv[:, 0:2 * P].rearrange("p (h d) -> p h d", h=HL),
                )

            def scores_exp(pair, ihalf, j):
                ps = pp.tile([P, 1024], F32, tag="s", name="s", bufs=2)
                for lane in range(2):
                    hp = lane * D
                    nc.tensor.matmul(
                        ps[:, lane * 512:(lane + 1) * 512],
                        lhsT=k_sb[pair][j // 4][hp:hp + D,
                                                (j % 4) * P:(j % 4 + 1) * P],
                        rhs=q_sb[pair][ihalf][hp:hp + D, :],
                        start=True, stop=True,
                    )
                e = epool.tile([P, 1024], F32R, tag="e", name="e")
                nc.scalar.activation(
                    out=e, in_=ps,
                    func=mybir.ActivationFunctionType.Exp,
                )
                return e

            def make_pv(py, pair, j, e):
                def fn():
                    for lane in range(2):
                        nc.tensor.matmul(
                            py[lane][0:D + 1, :],
                            lhsT=vt_sb[j][:, 2 * pair + lane, :],
                            rhs=e[:, lane * 512:(lane + 1) * 512],
                            start=(j == 0), stop=(j == J - 1),
                        )
                return fn

            def drain_copies(py, dd_on_act=False):
                # denominator rows first — they gate the selector matmul,
                # reciprocal and normalize chain.  At the tail one goes to
                # the (idle) ACT engine so both run concurrently.
                if dd_on_act:
                    nc.scalar.activation(
                        out=dd[0:1, :], in_=py[0][D:D + 1, :],
                        func=mybir.ActivationFunctionType.Identity)
                else:
                    nc.vector.tensor_copy(out=dd[0:1, :],
                                          in_=py[0][D:D + 1, :])
                nc.vector.tensor_copy(out=dd[32:33, :],
                                      in_=py[1][D:D + 1, :])
                # lane0: y rows at base 0
                yu0 = ypool.tile([D, 512], F32R, tag="yu", name="yu")
                nc.vector.tensor_copy(out=yu0, in_=py[0][0:D, :])
                # lane1: y rows shifted to the upper partition half so the
                # normalize + stacked out-projection stay base-aligned
                yu1 = ypool.tile([P, 512], F32R, tag="yu1", name="yu1")
                nc.vector.tensor_copy(out=yu1[D:2 * D, :],
                                      in_=py[1][0:D, :])
                return (yu0, yu1)

            def drain_norm(bi, pair, ihalf, pr_tag="po", all_dve=False):
                yu0, yu1 = st[bi]
                isl = slice(ihalf * 512, (ihalf + 1) * 512)
                # denominator broadcast: one K=33 selector matmul
                pr = pp.tile([P, 512], F32, tag=pr_tag, name="pr",
                             bufs=2 if pr_tag == "po" else 1)
                nc.tensor.matmul(
                    pr, lhsT=sel, rhs=dd, start=True, stop=True,
                )
                rr = rpool.tile([P, 512], F32, tag="rr", name="rr")
                nc.vector.reciprocal_approx_fast(out=rr, in_=pr)
                # normalize into the pair-stacked y tile; the two lanes go
                # to different engines so they run concurrently
                eng0 = nc.vector if all_dve else nc.gpsimd
                eng0.tensor_tensor(
                    out=yt[pair][0:D, isl],
                    in0=yu0, in1=rr[0:D, :],
                    op=mybir.AluOpType.mult,
                )
                nc.vector.tensor_tensor(
                    out=yt[pair][D:2 * D, isl],
                    in0=yu1[D:2 * D, :], in1=rr[D:2 * D, :],
                    op=mybir.AluOpType.mult,
                )

            def outproj_mm(m, g, po, isl):
                # stacked K=128 contraction sums the head pair directly
                nc.tensor.matmul(
                    po,
                    lhsT=wot_sb[:, g, m * P:(m + 1) * P],
                    rhs=yt[g][:, isl],
                    start=(g == 0), stop=(g == 1),
                )

            def qk(w_idx, m, nn, split):
                return lambda: proj_group(w_idx, m, nn, split=split)

            def vp(j):
                return lambda: vproj(j)

            ot0 = opool.tile([P, 4, 512], FP16, tag="ot")
            _d_po = {}

            def op0(m, phase):
                def fn():
                    if phase == 0:
                        _d_po[m] = po_tile()
                        outproj_mm(m, 0, _d_po[m], slice(0, 512))
                    elif phase == 1:
                        outproj_mm(m, 1, _d_po[m], slice(0, 512))
                    else:
                        nc.vector.tensor_copy(out=ot0[:, m, :],
                                              in_=_d_po[m])
                return fn

            st = {}
            norm = drain_norm
            blocks = [
                # (pair, ihalf, sched) — sched[j] runs between exp_j and
                # the (lagged) PV of the previous iteration.  Each block
                # interleaves only the projections it (or the next block)
                # needs soonest, balancing PE load across the exp stream.
                (0, 0, {
                    0: [vp(0), vp(1)], 1: [vp(2), qk(1, 0, 1, 0)],
                    2: [vp(3), qk(1, 0, 1, 1)], 3: [vp(4), qk(0, 0, 1, 0)],
                    4: [vp(5), qk(0, 0, 1, 1)], 5: [vp(6)], 6: [vp(7)],
                }),
                (0, 1, {
                    0: [qk(1, 1, 0, 0)], 1: [qk(1, 1, 0, 1)],
                    2: [lambda: norm(0, 0, 0), qk(0, 1, 0, 0)],
                    3: [qk(0, 1, 0, 1)],
                }),
                (1, 0, {
                    0: [qk(1, 1, 1, 0)], 1: [qk(1, 1, 1, 1)],
                    2: [lambda: norm(1, 0, 1), qk(0, 1, 1, 0)],
                    3: [qk(0, 1, 1, 1)],
                }),
                (1, 1, {
                    1: [lambda: norm(2, 1, 0)],
                    3: [op0(0, 0)], 4: [op0(0, 1), op0(0, 2)],
                    5: [op0(1, 0), op0(1, 1)],
                    6: [op0(1, 2), op0(2, 0), op0(2, 1)],
                    7: [op0(2, 2), op0(3, 0), op0(3, 1)],
                }),
            ]

            # ---- the flattened 32-iteration pipeline: extras lag one
            # iteration and PV lags two, so the scores/exp stream always
            # sits at the front of the in-order PE queue.
            pys = {}
            pend_extras = ()
            pend_pv = [None, None]
            for bi, (pair, ihalf, sched) in enumerate(blocks):
                pys[bi] = [
                    pp.tile([P, 512], F32, tag="py0", name="py0", bufs=1),
                    pp.tile([P, 512], F32, tag="py1", name="py1", bufs=1),
                ]
                for j in range(J):
                    e = scores_exp(pair, ihalf, j)
                    for fn in pend_extras:
                        fn()
                    pend_extras = sched.get(j, ())
                    if pend_pv[0] is not None:
                        pend_pv[0]()
                    pend_pv = [pend_pv[1], make_pv(pys[bi], pair, j, e)]
                    if j == 1 and bi > 0:
                        st[bi - 1] = drain_copies(pys[bi - 1])

            # ---- tail: i1 out-projection g=0 fills the PE while the last
            # PV + drain chain run; then g=1, copies, stores.
            out_t = out.rearrange("(m p) n -> p m n", p=P)
            pend_pv[0]()                      # PV of (D,6)
            for fn in pend_extras:            # D's j7 out-projection bits
                fn()
            tail_po = [po_tile() if m < 2 else po_tile(tag="s")
                       for m in range(4)]
            for m in range(4):
                outproj_mm(m, 0, tail_po[m], slice(512, 1024))
            pend_pv[1]()                      # PV of (D,7)
            st[3] = drain_copies(pys[3], dd_on_act=True)
            drain_norm(3, 1, 1, pr_tag="py0", all_dve=True)
            op0(3, 2)()
            nc.sync.dma_start(out=out_t[:, :, 0:512], in_=ot0)
            ot1 = opool.tile([P, 4, 512], FP16, tag="ot")
            for m in range(4):
                outproj_mm(m, 1, tail_po[m], slice(512, 1024))
                nc.vector.tensor_copy(out=ot1[:, m, :], in_=tail_po[m])
                nc.sync.dma_start(out=out_t[:, m, 512:1024],
                                  in_=ot1[:, m, :])

    nc.compile()
    return nc


def get_nc():
    if "nc" not in _NC_CACHE:
        _NC_CACHE["nc"] = build_nc()
    return _NC_CACHE["nc"]


def make_in_maps(x, Wq, Wk, Wv, Wo):
    in_maps = []
    for core in range(8):
        b, g = core // 2, core % 2
        sl = slice(g * 256, (g + 1) * 256)
        wqkv = np.stack(
            [Wq[sl, :].T, Wk[sl, :].T, Wv[sl, :].T], axis=1
        )  # (512, 3, 256)
        # paired Wo layout: wo_p[d + 64*(h%2), h//2, c_out]
        wo_l = Wo[:, sl].reshape(C, HL, D)          # (c_out, h, d)
        wo_p = np.empty((P, 2, C), dtype=np.float32)
        for h in range(HL):
            wo_p[(h % 2) * D:(h % 2) * D + D, h // 2, :] = wo_l[:, h, :].T
        in_maps.append({
            "x": np.ascontiguousarray(x[b]).astype(np.float16),
            "wqkv_t": np.ascontiguousarray(wqkv).astype(np.float16),
            "wo_p": np.ascontiguousarray(wo_p).astype(np.float16),
        })
    return in_maps


LAST_RESULTS = {}


def kernel(x, Wq, Wk, Wv, Wo, _trace=False):
    x = np.asarray(x, dtype=np.float32)
    Wq = np.asarray(Wq, dtype=np.float32)
    Wk = np.asarray(Wk, dtype=np.float32)
    Wv = np.asarray(Wv, dtype=np.float32)
    Wo = np.asarray(Wo, dtype=np.float32)

    nc = get_nc()
    in_maps = make_in_maps(x, Wq, Wk, Wv, Wo)
    res = run_bass_kernel_spmd(
        nc, in_maps, core_ids=list(range(8)), trace=_trace
    )
    LAST_RESULTS["res"] = res
    parts = [np.asarray(r["out_p"]).astype(np.float32) for r in res.results]
    out = np.stack([parts[2 * b] + parts[2 * b + 1] for b in range(4)])
    return out
